# revision 1
# baseline (speedup 1.0000x reference)
"""HGNN conv kernel for Trainium2, data-parallel over time across 8 cores.

Per core (t = core index): out_b = Dv^-1/2 Gc De^-1 Gc^T Dv^-1/2 (x_b W + 1 b^T)
computed in factored form (L never materialized):
  Gs  = Dv^-1/2 Gc                      [N, E]
  zT  = x_t^T Gs  per 128-row bf block  [BF, E]   (MM1)
  zw  = zT^T-blocks @ blockdiag(W,W) + u0 bias^T  [E, BF]  (W-MM + fused bias)
  out = Gsd^T zw with Gsd = de * Gs^T   [N, BF]   (MM2)
All matmuls run in float32r (full PE rate, ~1e-4 rel err).
"""

import sys

import numpy as np

sys.path.insert(0, "/opt/trn_rl_repo")

from contextlib import ExitStack

import concourse.bass as bass
import concourse.mybir as mybir
import concourse.tile as tile
from concourse import bacc, bass_utils
from concourse.masks import make_identity

P = 128
T = 8
B = 28          # batch entries per core
N = 1024        # nodes
E = 512         # hyperedges (256 static + 256 dynamic)
F = 64          # features
BF = B * F      # 1792
EPS = 1e-6
NT = N // P     # 8 n-tiles
ET = E // P     # 4 e-tiles
MT = BF // P    # 14 bf-tiles (2 batch entries each)
NB = 4          # output free-dim chunks
NBW = BF // NB  # 448 = 7 batch entries * 64

f32 = mybir.dt.float32
f32r = mybir.dt.float32r


def _build_nc():
    nc = bacc.Bacc("TRN2", target_bir_lowering=False, debug=False)

    xs = nc.dram_tensor("xs", [B, N, F], f32, kind="ExternalInput").ap()
    g = nc.dram_tensor("g", [N, 256], f32, kind="ExternalInput").ap()
    g1 = nc.dram_tensor("g1", [N, 256], f32, kind="ExternalInput").ap()
    w = nc.dram_tensor("w", [F, F], f32, kind="ExternalInput").ap()
    bvec = nc.dram_tensor("b", [F], f32, kind="ExternalInput").ap()
    os_ = nc.dram_tensor("os", [B, N, F], f32, kind="ExternalOutput").ap()

    with tile.TileContext(nc) as tc, ExitStack() as ctx:
        const = ctx.enter_context(tc.tile_pool(name="const", bufs=1))
        big = ctx.enter_context(tc.tile_pool(name="big", bufs=1))
        ztp = ctx.enter_context(tc.tile_pool(name="ztp", bufs=3))
        osb = ctx.enter_context(tc.tile_pool(name="osb", bufs=4))
        ps_stats = ctx.enter_context(tc.tile_pool(name="ps_stats", bufs=1, space="PSUM"))
        ps_small = ctx.enter_context(tc.tile_pool(name="ps_small", bufs=2, space="PSUM"))
        ps_z = ctx.enter_context(tc.tile_pool(name="ps_z", bufs=2, space="PSUM"))
        ps_o = ctx.enter_context(tc.tile_pool(name="ps_o", bufs=2, space="PSUM"))

        # ---- input loads -------------------------------------------------
        # x slice as [n-part, k(n-tile), b, f], cast to f32r during DMA
        xs_all = big.tile([P, NT, B, F], f32r, name="xs_all")
        xs_r = xs.rearrange("b (k p) f -> p k b f", p=P)
        for k in range(NT):
            nc.gpsimd.dma_start(xs_all[:, k], xs_r[:, k])

        # Gc = [G | G1] as [n-part, k, e], cast to f32r during DMA
        gc_all = big.tile([P, NT, E], f32r, name="gc_all")
        nc.gpsimd.dma_start(gc_all[:, :, 0:256], g.rearrange("(k p) e -> p k e", p=P))
        nc.gpsimd.dma_start(gc_all[:, :, 256:512], g1.rearrange("(k p) e -> p k e", p=P))

        # blockdiag(W, W) [128, 128] f32r
        bdw_f = const.tile([P, P], f32, name="bdw_f")
        nc.vector.memset(bdw_f[:], 0.0)
        nc.sync.dma_start(bdw_f[0:64, 0:64], w)
        nc.sync.dma_start(bdw_f[64:128, 64:128], w)
        bdw = const.tile([P, P], f32r, name="bdw")
        nc.vector.tensor_copy(bdw[:], bdw_f[:])

        # bias tiled twice [1, 128] f32r
        btmp = const.tile([1, F], f32, name="btmp")
        nc.sync.dma_start(btmp[:], bvec[None, :])
        bias2 = const.tile([1, 2, F], f32r, name="bias2")
        nc.vector.tensor_copy(bias2[:], btmp[0:1, None, :].to_broadcast([1, 2, F]))
        bias_bc = const.tile([P, P], f32r, name="bias_bc")
        nc.gpsimd.partition_broadcast(
            bias_bc[:], bias2[:].rearrange("o t f -> o (t f)")
        )

        ident_f = const.tile([P, P], f32, name="ident_f")
        make_identity(nc, ident_f[:])
        ident = const.tile([P, P], f32r, name="ident")
        nc.vector.tensor_copy(ident[:], ident_f[:])

        # ---- degree stats ------------------------------------------------
        # dv = 1/sqrt(rowsum(Gc) + eps)   [128, NT]
        rs = const.tile([P, NT], f32, name="rs")
        for k in range(NT):
            nc.vector.reduce_sum(rs[:, k : k + 1], gc_all[:, k, :], axis=mybir.AxisListType.X)
        eps_col = const.tile([P, 1], f32, name="eps_col")
        nc.vector.memset(eps_col[:], EPS)
        sq = const.tile([P, NT], f32, name="sq")
        nc.scalar.activation(
            sq[:], rs[:], mybir.ActivationFunctionType.Sqrt, bias=eps_col[:]
        )
        dv = const.tile([P, NT], f32, name="dv")
        nc.vector.reciprocal(dv[:], sq[:])

        # lhsT per k-tile: [ones | dv_k] -> colsums of Gc (row 0) and Gs (row 1)
        onesdv_f = const.tile([P, NT, 2], f32, name="onesdv_f")
        nc.vector.memset(onesdv_f[:, :, 0:1], 1.0)
        nc.vector.tensor_copy(onesdv_f[:, :, 1:2], dv[:, :, None])
        onesdv = const.tile([P, NT, 2], f32r, name="onesdv")
        nc.vector.tensor_copy(onesdv[:], onesdv_f[:])
        stats_ps = ps_stats.tile([2, E], f32, name="stats_ps")
        for k in range(NT):
            nc.tensor.matmul(
                stats_ps[:], onesdv[:, k, :], gc_all[:, k, :],
                start=(k == 0), stop=(k == NT - 1),
            )
        stats_sb = const.tile([2, E], f32r, name="stats_sb")
        nc.vector.tensor_copy(stats_sb[:], stats_ps[:])

        # transpose stats to column layout [128, ET, 2] = [cs | u0]
        statsT = const.tile([P, ET, 2], f32, name="statsT")
        for j in range(ET):
            tp = ps_small.tile([P, P], f32r, name="sp")[:, 0:2]
            nc.tensor.matmul(
                tp[:], stats_sb[:, j * P : (j + 1) * P], ident[0:2, 0:2],
                is_transpose=True,
            )
            nc.vector.tensor_copy(statsT[:, j, :], tp[:])
        de_col = const.tile([P, ET], f32, name="de_col")
        nc.vector.tensor_scalar(
            out=de_col[:], in0=statsT[:, :, 0], scalar1=EPS, scalar2=None,
            op0=mybir.AluOpType.add,
        )
        nc.vector.reciprocal(de_col[:], de_col[:])

        # ---- Gs and Gsd --------------------------------------------------
        gs_all = big.tile([P, NT, E], f32r, name="gs_all")
        for k in range(NT):
            nc.vector.tensor_scalar(
                out=gs_all[:, k, :], in0=gc_all[:, k, :], scalar1=dv[:, k : k + 1],
                scalar2=None, op0=mybir.AluOpType.mult,
            )

        # Gsd[e, n] = de[e] * Gs[n, e] via PE transpose + scaled evict
        gsd_all = big.tile([P, ET, N], f32r, name="gsd_all")
        for j in range(ET):
            for i in range(NT):
                tp = ps_small.tile([P, P], f32r, name="sp")
                nc.tensor.matmul(
                    tp[:], gs_all[:, i, j * P : (j + 1) * P], ident[:],
                    is_transpose=True,
                )
                nc.vector.tensor_scalar(
                    out=gsd_all[:, j, i * P : (i + 1) * P], in0=tp[:],
                    scalar1=de_col[:, j : j + 1], scalar2=None,
                    op0=mybir.AluOpType.mult,
                )

        # ---- MM1 + W-MM pipeline ----------------------------------------
        # v_all[e-part, j, bf'] = de-unscaled zw + u0*bias  (f32r)
        v_all = big.tile([P, ET, BF], f32r, name="v_all")
        xs_flat = xs_all[:].rearrange("p k b f -> p k (b f)")

        for m in range(MT):
            zps = ps_z.tile([P, E], f32, name="zps")
            for k in range(NT):
                nc.tensor.matmul(
                    zps[:], xs_flat[:, k, m * P : (m + 1) * P], gs_all[:, k, :],
                    start=(k == 0), stop=(k == NT - 1),
                )
            zt = ztp.tile([P, E], f32r, name="zt")
            nc.scalar.copy(zt[:], zps[:])
            for j in range(ET):
                wps = ps_small.tile([P, P], f32, name="sp")
                nc.tensor.matmul(
                    wps[:], zt[:, j * P : (j + 1) * P], bdw[:],
                    start=True, stop=True,
                )
                # v = (bias_bcast * u0_col) + zw_psum, rounded to f32r
                nc.vector.scalar_tensor_tensor(
                    out=v_all[:, j, m * P : (m + 1) * P],
                    in0=bias_bc[:],
                    scalar=statsT[:, j, 1:2],
                    in1=wps[:],
                    op0=mybir.AluOpType.mult,
                    op1=mybir.AluOpType.add,
                )

        # ---- MM2 + store -------------------------------------------------
        os_r = os_.rearrange("b (i p) f -> p i b f", p=P)
        for i in range(NT):
            for nb in range(NB):
                ops = ps_o.tile([P, NBW], f32, name="ops")
                for j in range(ET):
                    nc.tensor.matmul(
                        ops[:], gsd_all[:, j, i * P : (i + 1) * P],
                        v_all[:, j, nb * NBW : (nb + 1) * NBW],
                        start=(j == 0), stop=(j == ET - 1),
                    )
                ot = osb.tile([P, NBW], f32, name="ot")
                if (i * NB + nb) % 2 == 0:
                    nc.scalar.copy(ot[:], ops[:])
                else:
                    nc.vector.tensor_copy(ot[:], ops[:])
                nc.sync.dma_start(
                    os_r[:, i, nb * 7 : (nb + 1) * 7, :],
                    ot[:].rearrange("p (c f) -> p c f", f=F),
                )

    nc.finalize()
    return nc


_NC = None


def _get_nc():
    global _NC
    if _NC is None:
        _NC = _build_nc()
    return _NC


def kernel(x, G, G1, weight, bias):
    nc = _get_nc()
    x = np.ascontiguousarray(x, dtype=np.float32)
    G = np.ascontiguousarray(G, dtype=np.float32)
    G1 = np.ascontiguousarray(G1, dtype=np.float32)
    weight = np.ascontiguousarray(weight, dtype=np.float32)
    bias = np.ascontiguousarray(bias, dtype=np.float32)

    in_maps = []
    for c in range(T):
        in_maps.append(
            {
                "xs": x[c * B : (c + 1) * B],
                "g": G,
                "g1": np.ascontiguousarray(G1[c]),
                "w": weight,
                "b": bias,
            }
        )
    res = bass_utils.run_bass_kernel_spmd(nc, in_maps, core_ids=list(range(T)))
    return np.concatenate([r["os"] for r in res.results], axis=0)



# revision 2
# speedup vs baseline: 1.5772x; 1.5772x over previous
"""HGNN conv kernel for Trainium2, data-parallel over time across 8 cores.

Per core (t = core index): out = Dv^-1/2 Gc De^-1 Gc^T Dv^-1/2 (x W + 1 b^T),
computed in factored form (L never materialized):
  Gs   = dv * Gc                      [N, E]   (dv = rsqrt(rowsum Gc))
  z^T  = x^T Gs  per 128-row bf block [BF, E]  (MM1, bf16)
  v    = z W + u0 bias^T              [E, BF]  (W-MM transposes + applies W)
  out  = dv * (Gsd^T v), Gsd = de*Gc^T [N, BF] (MM2; dv folded into evict)

Host-side (layout only): x transposed to [N, B*F] per core, Gc^T
pretransposed, everything cast to bf16 so DMA descriptors are multi-KB
contiguous runs and stationary weights get FastWeightLoad. Output is
written as [N, B*F] f32 and transposed back on the host.
"""

import sys

import numpy as np

sys.path.insert(0, "/opt/trn_rl_repo")

from contextlib import ExitStack

import ml_dtypes

import concourse.bass as bass
import concourse.mybir as mybir
import concourse.tile as tile
from concourse import bacc, bass_utils
from concourse.masks import make_identity

P = 128
T = 8
B = 28          # batch entries per core
N = 1024        # nodes
E = 512         # hyperedges (256 static + 256 dynamic)
F = 64          # features
BF = B * F      # 1792
EPS = 1e-6
NT = N // P     # 8 n-tiles
ET = E // P     # 4 e-tiles
MT = BF // P    # 14 bf-tiles (2 batch entries each)
NB = 4          # output free-dim chunks
NBW = BF // NB  # 448

f32 = mybir.dt.float32
f32r = mybir.dt.float32r
bf16 = mybir.dt.bfloat16
BF16 = ml_dtypes.bfloat16


def _build_nc():
    nc = bacc.Bacc("TRN2", target_bir_lowering=False, debug=False)

    xs = nc.dram_tensor("xs", [N, BF], bf16, kind="ExternalInput").ap()
    gc = nc.dram_tensor("gc", [N, E], bf16, kind="ExternalInput").ap()
    gct = nc.dram_tensor("gct", [E, N], bf16, kind="ExternalInput").ap()
    bdw = nc.dram_tensor("bdw", [P, P], bf16, kind="ExternalInput").ap()
    b2 = nc.dram_tensor("b2", [1, P], f32, kind="ExternalInput").ap()
    os_ = nc.dram_tensor("os", [N, BF], f32, kind="ExternalOutput").ap()

    with tile.TileContext(nc) as tc, ExitStack() as ctx:
        const = ctx.enter_context(tc.tile_pool(name="const", bufs=1))
        big = ctx.enter_context(tc.tile_pool(name="big", bufs=1))
        ztp = ctx.enter_context(tc.tile_pool(name="ztp", bufs=3))
        osb = ctx.enter_context(tc.tile_pool(name="osb", bufs=3))
        ps_stats = ctx.enter_context(tc.tile_pool(name="ps_stats", bufs=1, space="PSUM"))
        ps_small = ctx.enter_context(tc.tile_pool(name="ps_small", bufs=2, space="PSUM"))
        ps_z = ctx.enter_context(tc.tile_pool(name="ps_z", bufs=2, space="PSUM"))
        ps_o = ctx.enter_context(tc.tile_pool(name="ps_o", bufs=2, space="PSUM"))

        # ---- input loads (all contiguous multi-KB per partition) ----------
        gc_all = big.tile([P, NT, E], bf16, name="gc_all")
        nc.gpsimd.dma_start(gc_all[:], gc.rearrange("(k p) e -> p k e", p=P))
        gct_all = big.tile([P, ET, N], bf16, name="gct_all")
        nc.gpsimd.dma_start(gct_all[:], gct.rearrange("(j p) n -> p j n", p=P))

        xs_all = big.tile([P, NT, BF], bf16, name="xs_all")
        xs_r = xs.rearrange("(k p) m -> p k m", p=P)
        for k in range(NT):
            nc.gpsimd.dma_start(xs_all[:, k], xs_r[:, k])

        bdw_sb = const.tile([P, P], bf16, name="bdw_sb")
        nc.sync.dma_start(bdw_sb[:], bdw)
        b2_sb = const.tile([1, P], f32, name="b2_sb")
        nc.sync.dma_start(b2_sb[:], b2)
        bias_bc = const.tile([P, P], f32, name="bias_bc")
        nc.gpsimd.partition_broadcast(bias_bc[:], b2_sb[:])

        ident_f = const.tile([P, P], f32, name="ident_f")
        make_identity(nc, ident_f[:])
        ident = const.tile([P, P], f32r, name="ident")
        nc.vector.tensor_copy(ident[:], ident_f[:])

        # ---- degree stats -------------------------------------------------
        # dv = 1/sqrt(rowsum(Gc) + eps)   [128, NT]
        rs = const.tile([P, NT, 1], f32, name="rs")
        nc.vector.reduce_sum(rs[:], gc_all[:], axis=mybir.AxisListType.X)
        eps_col = const.tile([P, 1], f32, name="eps_col")
        nc.vector.memset(eps_col[:], EPS)
        sq = const.tile([P, NT], f32, name="sq")
        nc.scalar.activation(
            sq[:], rs[:, :, 0], mybir.ActivationFunctionType.Sqrt, bias=eps_col[:]
        )
        dv = const.tile([P, NT], f32, name="dv")
        nc.vector.reciprocal(dv[:], sq[:])

        # stats rows: [ones | dv] x Gc -> row0 = colsum(Gc), row1 = colsum(Gs)
        onesdv_f = const.tile([P, NT, 2], f32, name="onesdv_f")
        nc.vector.memset(onesdv_f[:, :, 0:1], 1.0)
        nc.vector.tensor_copy(onesdv_f[:, :, 1:2], dv[:, :, None])
        onesdv = const.tile([P, NT, 2], bf16, name="onesdv")
        nc.vector.tensor_copy(onesdv[:], onesdv_f[:])
        stats_ps = ps_stats.tile([2, E], f32, name="stats_ps")
        for k in range(NT):
            nc.tensor.matmul(
                stats_ps[:], onesdv[:, k, :], gc_all[:, k, :],
                start=(k == 0), stop=(k == NT - 1),
            )
        stats_sb = const.tile([2, E], f32r, name="stats_sb")
        nc.vector.tensor_copy(stats_sb[:], stats_ps[:])

        # transpose stats to column layout [128, ET, 2] = [cs | u0]
        statsT = const.tile([P, ET, 2], f32, name="statsT")
        for j in range(ET):
            tp = ps_small.tile([P, ET, P], f32r, name="sp")[:, 0, 0:2]
            nc.tensor.matmul(
                tp, stats_sb[:, j * P : (j + 1) * P], ident[0:2, 0:2],
                is_transpose=True,
            )
            nc.vector.tensor_copy(statsT[:, j, :], tp)
        de_col = const.tile([P, ET], f32, name="de_col")
        nc.vector.tensor_scalar(
            out=de_col[:], in0=statsT[:, :, 0], scalar1=EPS, scalar2=None,
            op0=mybir.AluOpType.add,
        )
        nc.vector.reciprocal(de_col[:], de_col[:])

        # ---- scaled matrices ---------------------------------------------
        gs_all = big.tile([P, NT, E], bf16, name="gs_all")
        for k in range(NT):
            nc.vector.tensor_scalar(
                out=gs_all[:, k, :], in0=gc_all[:, k, :], scalar1=dv[:, k : k + 1],
                scalar2=None, op0=mybir.AluOpType.mult,
            )
        gsd_all = big.tile([P, ET, N], bf16, name="gsd_all")
        for j in range(ET):
            nc.vector.tensor_scalar(
                out=gsd_all[:, j, :], in0=gct_all[:, j, :],
                scalar1=de_col[:, j : j + 1], scalar2=None,
                op0=mybir.AluOpType.mult,
            )
        # bias_u0[e, (b2,f)] = u0[e] * bias2[(b2,f)]
        bias_u0 = const.tile([P, ET, P], f32, name="bias_u0")
        for j in range(ET):
            nc.vector.tensor_scalar(
                out=bias_u0[:, j, :], in0=bias_bc[:],
                scalar1=statsT[:, j, 1:2], scalar2=None,
                op0=mybir.AluOpType.mult,
            )

        # ---- MM1 + W-MM pipeline -----------------------------------------
        v_all = big.tile([P, ET, BF], bf16, name="v_all")
        for m in range(MT):
            zps = ps_z.tile([P, E], f32, name="zps")
            for k in range(NT):
                nc.tensor.matmul(
                    zps[:], xs_all[:, k, m * P : (m + 1) * P], gs_all[:, k, :],
                    start=(k == 0), stop=(k == NT - 1),
                )
            zt = ztp.tile([P, E], bf16, name="zt")
            nc.scalar.copy(zt[:], zps[:])
            wps = ps_small.tile([P, ET, P], f32, name="sp")
            for j in range(ET):
                nc.tensor.matmul(
                    wps[:, j, :], zt[:, j * P : (j + 1) * P], bdw_sb[:],
                    start=True, stop=True,
                )
            # v = bias_u0 + zw, rounded to bf16 (one instr per m-tile)
            nc.vector.scalar_tensor_tensor(
                out=v_all[:, :, m * P : (m + 1) * P],
                in0=bias_u0[:],
                scalar=1.0,
                in1=wps[:],
                op0=mybir.AluOpType.mult,
                op1=mybir.AluOpType.add,
            )

        # ---- MM2 + store --------------------------------------------------
        os_r = os_.rearrange("(i p) m -> p i m", p=P)
        for i in range(NT):
            ost = osb.tile([P, BF], f32, name="ost")
            for nb in range(NB):
                ops = ps_o.tile([P, NBW], f32, name="ops")
                for j in range(ET):
                    nc.tensor.matmul(
                        ops[:], gsd_all[:, j, i * P : (i + 1) * P],
                        v_all[:, j, nb * NBW : (nb + 1) * NBW],
                        start=(j == 0), stop=(j == ET - 1),
                    )
                # out = dv[n] * psum (alternate engines)
                dst = ost[:, nb * NBW : (nb + 1) * NBW]
                if nb % 2 == 0:
                    nc.vector.tensor_scalar(
                        out=dst, in0=ops[:], scalar1=dv[:, i : i + 1],
                        scalar2=None, op0=mybir.AluOpType.mult,
                    )
                else:
                    nc.scalar.mul(dst, ops[:], dv[:, i : i + 1])
            nc.sync.dma_start(os_r[:, i], ost[:])

    nc.finalize()
    return nc


_NC = None


def _get_nc():
    global _NC
    if _NC is None:
        _NC = _build_nc()
    return _NC


def _in_maps(x, G, G1, weight, bias):
    x = np.ascontiguousarray(x, dtype=np.float32)
    G = np.ascontiguousarray(G, dtype=np.float32)
    G1 = np.ascontiguousarray(G1, dtype=np.float32)
    weight = np.ascontiguousarray(weight, dtype=np.float32)
    bias = np.ascontiguousarray(bias, dtype=np.float32)

    # x: [T*B, N, F] -> per core [N, B*F], bf16
    xh = np.ascontiguousarray(
        x.reshape(T, B, N, F).transpose(0, 2, 1, 3)
    ).reshape(T, N, BF).astype(BF16)
    # Gc = [G | G1[t]] and its transpose, bf16
    gc_np = np.concatenate(
        [np.broadcast_to(G[None], (T, N, 256)), G1], axis=2
    )
    gch = gc_np.astype(BF16)
    gcth = np.ascontiguousarray(gc_np.transpose(0, 2, 1)).astype(BF16)
    # blockdiag(W, W) built on host
    bdw_h = np.zeros((P, P), dtype=BF16)
    bdw_h[:F, :F] = weight.astype(BF16)
    bdw_h[F:, F:] = weight.astype(BF16)
    b2_h = np.tile(bias, 2).reshape(1, P).astype(np.float32)

    maps = []
    for c in range(T):
        maps.append(
            {
                "xs": xh[c],
                "gc": np.ascontiguousarray(gch[c]),
                "gct": gcth[c],
                "bdw": bdw_h,
                "b2": b2_h,
            }
        )
    return maps


def kernel(x, G, G1, weight, bias):
    nc = _get_nc()
    res = bass_utils.run_bass_kernel_spmd(
        nc, _in_maps(x, G, G1, weight, bias), core_ids=list(range(T))
    )
    # os: per core [N, B*F] -> [T, N, B, F] -> [T, B, N, F] -> [bs, N, F]
    out = np.stack([r["os"] for r in res.results], axis=0)
    return np.ascontiguousarray(
        out.reshape(T, N, B, F).transpose(0, 2, 1, 3)
    ).reshape(T * B, N, F)
